# revision 13
# baseline (speedup 1.0000x reference)
"""GAT-style 3-layer attention graph network on 8 TRN2 NeuronCores.

Math: per layer, alpha[i,j] = adj[i,j]*exp(el[i]+er[j]+ab) / sum_k adj[i,k]*exp(el[i]+er[k]+ab)
The exp(el[i]) factor cancels between numerator and denominator, so with
w[j] = exp(er[j]+ab):
    out[i] = relu( (sum_j adj[i,j]*w[j]*h[j]) / (sum_j adj[i,j]*w[j]) )
i.e. one [N,N]@[N,F+1] matmul per layer against G = [h*w | w], with adj
constant across layers.

Distribution: row-shard adj across the 8 cores (1024 dest rows each). adj is
0/1 so it is exactly representable in fp8_e4m3: the host pre-transposes each
core's row-block into the matmul lhsT tile layout [128, m, k, 128] fp8
(the PE contracts over the partition index, which for the aggregation is
adj's column index), and it stays SBUF-resident (8MB/core) across all 3
layers; the mixed fp8-lhsT x fp16-rhs matmul is exact for 0/1 weights.
Each layer all-gathers the 8192x(F+1) fp16 G matrix (2MB) in two node-halves
so the first gather overlaps the previous layer's aggregation, and the next
layer's G is built inside the per-m epilogue of the current aggregation.
"""
import numpy as np

import concourse.bass as bass
import concourse.mybir as mybir
import concourse.tile as tile
from concourse.masks import make_identity
from concourse.bass_utils import run_bass_kernel_spmd

F32 = mybir.dt.float32
F16 = mybir.dt.float16  # G storage dtype: 10-bit mantissa
F8 = mybir.dt.float8e4   # adj storage: 0/1 exact in fp8_e4m3, 4x weight-load

N_CORES = 8
N = 8192
NL = N // N_CORES          # 1024 local dest rows per core
NT = NL // 128             # 8 local node tiles
KT = N // 128              # 64 contraction tiles
LEAK = 0.2


def _split_excess_waits(nc, max_waits=1):
    """This walrus build allows only one sync-wait command per instruction;
    split any instruction carrying more into preceding single-wait nops."""
    n_split = 0
    for fn in nc.m.functions:
        for bb in fn.blocks:
            insts = bb.instructions
            i = 0
            while i < len(insts):
                inst = insts[i]
                si = inst.sync_info
                if si is not None and len(si.on_wait) > max_waits:
                    waits = list(si.on_wait)
                    extra, keep = waits[:-max_waits], waits[-max_waits:]
                    nops = []
                    for j, w in enumerate(extra):
                        nop = mybir.InstNoOp(
                            name=f"{inst.name}-waitsplit-{j}", ins=[], outs=[]
                        )
                        nop.engine = inst.engine
                        nop.sync_info = mybir.SyncInfo(on_wait=[w], on_update=[])
                        nops.append(nop)
                    inst.sync_info = mybir.SyncInfo(
                        on_wait=keep, on_update=list(si.on_update)
                    )
                    insts[i:i] = nops
                    i += len(nops)
                    n_split += 1
                i += 1
    return n_split


def _build_program(ab, for_sim=False):
    """ab: the three attention bias floats (baked in as memset constants)."""
    fhs = [128, 128, 64]  # per-layer linear output width

    nc = bass.Bass(num_devices=N_CORES)

    adj_ext = nc.dram_tensor("adjt", [128, NT, KT, 128], F8, kind="ExternalInput")
    x_ext = nc.dram_tensor("xt_local", [128, NL], F32, kind="ExternalInput")
    # packed params: cols [0:128)=w0t [128:256)=w1t [256:320)=w2t,
    # 320+l = b_l column, 323+l = awr_l column (rows past fh zero-padded)
    par_ext = nc.dram_tensor("params", [128, 326], F32, kind="ExternalInput")
    out_ext = nc.dram_tensor("out", [NL, 64], F32, kind="ExternalOutput")

    # all-gather payload in tiled layout, split in two node-halves per layer
    # so the first gather overlaps the previous aggregation: half h of layer l
    # holds rank blocks [128, 4*(fh+1)] with (p, t, f) = G[c*1024+(4h+t)*128+p, f]
    ag_ext = [[nc.dram_tensor(f"ag{l}h{h}", [N_CORES * 128, NT // 2 * (fhs[l] + 1)],
                              F16, addr_space="Shared") for h in range(2)]
              for l in range(3)]

    with tile.TileContext(nc) as tc:
        with (
            tc.tile_pool(name="const", bufs=1) as cp,
            tc.tile_pool(name="adjt", bufs=1) as ap_,
            tc.tile_pool(name="slabs", bufs=3) as sp,
            tc.tile_pool(name="gsb", bufs=2) as gp,
            tc.tile_pool(name="misc", bufs=2) as mp,
            tc.tile_pool(name="gloc", bufs=2) as glp,
            tc.tile_pool(name="dram", bufs=3, space="DRAM") as dp,
            tc.tile_pool(name="ptf32", bufs=2, space="PSUM") as ptf32,
            tc.tile_pool(name="plin", bufs=2, space="PSUM") as plin,
            tc.tile_pool(name="per", bufs=2, space="PSUM") as per,
            tc.tile_pool(name="pbig", bufs=2, space="PSUM") as pbig,
        ):
            # ---- constants / params ----
            ident_f32 = cp.tile([128, 128], F32)
            make_identity(nc, ident_f32[:])
            par = cp.tile([128, 326], F32)
            nc.sync.dma_start(out=par[:], in_=par_ext.ap())
            woff = [0, 128, 256]
            wt_sb = [par[:, woff[l]:woff[l] + fhs[l]] for l in range(3)]
            b_sb = [par[0:fhs[l], 320 + l:321 + l] for l in range(3)]
            awr_sb = [par[0:fhs[l], 323 + l:324 + l] for l in range(3)]
            ab_sb = []
            for l in range(3):
                t = cp.tile([128, 1], F32, tag=f"ab{l}")
                nc.gpsimd.memset(t[:], float(ab[l]))
                ab_sb.append(t)

            # ---- x arrives pre-transposed: [fi, node] ----
            curT = sp.tile([128, NL], F32, tag="slab")
            nc.sync.dma_start(out=curT[:], in_=x_ext.ap())

            # ---- adj arrives pre-transposed+tiled from host: [128, m, k, 128]
            # f16; tile (k, m) = adj[m-block rows, k-block cols].T. Load in
            # m-pair chunks so layer-0 m-chains can start after ~1/4 the DMA.
            adjT = ap_.tile([128, NT, KT, 128], F8)
            for d in range(NT // 2):
                nc.gpsimd.dma_start(
                    out=adjT[:, d * 2:(d + 1) * 2, :, :],
                    in_=adj_ext[:, d * 2:(d + 1) * 2, :, :],
                )

            # ---- G-prep helper: one 128-node block of layer l's G ----
            # src_col: [128(fi), 128] column of transposed prev activations
            def prep_block(l, src_col, gl, m):
                fh = fhs[l]
                pl = plin.tile([128, 128], F32, tag="lin")
                nc.tensor.matmul(pl[0:fh, 0:128], wt_sb[l], src_col,
                                 start=True, stop=True)
                hcol = mp.tile([128, 128], F32, tag="hcol")
                nc.scalar.activation(
                    hcol[0:fh, :], pl[0:fh, 0:128],
                    mybir.ActivationFunctionType.Prelu,
                    bias=b_sb[l], scale=1.0, alpha=LEAK,
                )
                pe_ = per.tile([128, 1], F32, tag="er")
                nc.tensor.matmul(pe_[:, 0:1], hcol[0:fh, :], awr_sb[l],
                                 start=True, stop=True)
                ec = mp.tile([128, 1], F32, tag="expc")
                nc.scalar.activation(
                    ec[:], pe_[:, 0:1], mybir.ActivationFunctionType.Exp,
                    bias=ab_sb[l][:], scale=1.0,
                )
                ptg = ptf32.tile([128, 128], F32, tag="ptf")
                nc.tensor.transpose(ptg[:, 0:fh], hcol[0:fh, :],
                                    ident_f32[0:fh, 0:fh])
                nc.vector.tensor_scalar_mul(gl[:, m, 0:fh], ptg[:, 0:fh], ec[:])
                nc.vector.tensor_copy(gl[:, m, fh:fh + 1], ec[:])

            def fire_gather(l, gl, h):
                """All-gather node-half h of layer l's local G block."""
                fh = fhs[l]
                gld = dp.tile([128, NT // 2 * (fh + 1)], F16, tag="gld")
                nc.sync.dma_start(
                    out=gld[:], in_=gl[:, h * (NT // 2):(h + 1) * (NT // 2), :]
                )
                if for_sim:
                    nc.sync.dma_start(out=ag_ext[l][h][0:128, :], in_=gld[:])
                else:
                    nc.gpsimd.collective_compute(
                        "AllGather", mybir.AluOpType.bypass,
                        replica_groups=[list(range(N_CORES))],
                        ins=[gld.opt()], outs=[ag_ext[l][h].ap().opt()],
                    )

            # ---- layer 0 G from x (overlaps the adj load) ----
            gl_cur = glp.tile([128, NT, fhs[0] + 1], F16, tag="gloc")
            for m in range(NT):
                prep_block(0, curT[:, m * 128:(m + 1) * 128], gl_cur, m)
                if m == NT // 2 - 1:
                    fire_gather(0, gl_cur, 0)
            fire_gather(0, gl_cur, 1)

            # ---- layers: all-gather G, aggregate, and build next layer's G
            # inside the per-m epilogue so only the collective + G reload sit
            # on the layer boundary ----
            for l in range(3):
                fh = fhs[l]
                # load the two gathered node-halves; k-tile k = c*NT + t, so
                # half h covers k with (k % NT) in [4h, 4h+4)
                gsb = gp.tile([128, N_CORES, NT, fh + 1], F16, tag="gsb")
                for hh in range(2):
                    nc.sync.dma_start(
                        out=gsb[:, :, hh * (NT // 2):(hh + 1) * (NT // 2), :],
                        in_=ag_ext[l][hh].ap().rearrange(
                            "(c p) (t f) -> p c t f", p=128, f=fh + 1
                        ),
                    )
                if l < 2:
                    gl_next = glp.tile([128, NT, fhs[l + 1] + 1], F16, tag="gloc")
                else:
                    ostage = mp.tile([128, NT, 64], F32, tag="ostage")

                # epilogue (+ next-layer G prep) for block m, emitted one
                # m-iteration late so the PE's static order interleaves the
                # small prep ops between big-MM chains without stalling
                def epilogue(m, bp):
                    recip = mp.tile([128, 1], F32, tag="recip")
                    nc.vector.reciprocal(recip[:], bp[:, fh:fh + 1])
                    if l < 2:
                        h2 = mp.tile([128, fh], F32, tag="h2")
                        nc.scalar.activation(
                            h2[:], bp[:, 0:fh], mybir.ActivationFunctionType.Relu,
                            bias=0.0, scale=recip[:],
                        )
                        pt = ptf32.tile([128, 128], F32, tag="ptf")
                        nc.tensor.transpose(pt[:, 0:128], h2[:], ident_f32[:])
                        cpcol = mp.tile([128, 128], F32, tag="cpcol")
                        nc.vector.tensor_copy(cpcol[:], pt[:, 0:128])
                        prep_block(l + 1, cpcol[:], gl_next, m)
                    else:
                        nc.scalar.activation(
                            ostage[:, m, :], bp[:, 0:fh],
                            mybir.ActivationFunctionType.Relu,
                            bias=0.0, scale=recip[:],
                        )

                ks = [k for k in range(KT) if k % NT < NT // 2] + \
                     [k for k in range(KT) if k % NT >= NT // 2]
                pending = None
                for m in range(NT):
                    bp = pbig.tile([128, fh + 1], F32, tag="big")
                    for i, k in enumerate(ks):
                        nc.tensor.matmul(
                            bp[:],
                            adjT[:, m, k, :],
                            gsb[:, k // NT, k % NT, :],
                            start=(i == 0), stop=(i == KT - 1),
                        )
                    if pending is not None:
                        epilogue(*pending)
                        if l < 2 and pending[0] == NT // 2 - 1:
                            fire_gather(l + 1, gl_next, 0)
                    pending = (m, bp)
                epilogue(*pending)
                if l < 2:
                    fire_gather(l + 1, gl_next, 1)
                    gl_cur = gl_next
                else:
                    nc.sync.dma_start(
                        out=out_ext.ap().rearrange("(m p) f -> p m f", p=128),
                        in_=ostage[:],
                    )

    _split_excess_waits(nc)
    return nc


_PROG_CACHE = {}


def _get_program(ab):
    key = tuple(round(a, 9) for a in ab)
    if key not in _PROG_CACHE:
        _PROG_CACHE[key] = _build_program(ab)
    return _PROG_CACHE[key]


def _make_in_maps(inputs):
    """Build the per-core input maps from the full (unsharded) input dict."""
    fhs = [128, 128, 64]
    x = np.asarray(inputs["x"], np.float32)
    adj = np.asarray(inputs["adj"], np.float32)
    in_maps = []
    for c in range(N_CORES):
        import ml_dtypes
        blk = adj[c * NL:(c + 1) * NL, :].astype(ml_dtypes.float8_e4m3)
        # [NL, N] -> [m, q, k, p] -> lhsT tile layout [p, m, k, q]
        adjt = blk.reshape(NT, 128, KT, 128).transpose(3, 0, 2, 1)
        m = {
            "adjt": np.ascontiguousarray(adjt),
            "xt_local": np.ascontiguousarray(x[c * NL:(c + 1) * NL, :].T),
        }
        par = np.zeros((128, 326), np.float32)
        woff = [0, 128, 256]
        for l in range(3):
            W = np.asarray(inputs[f"W{l}"], np.float32)
            b = np.asarray(inputs[f"b{l}"], np.float32)
            aW = np.asarray(inputs[f"aW{l}"], np.float32)
            par[:, woff[l]:woff[l] + fhs[l]] = W.T
            par[:fhs[l], 320 + l] = b.reshape(-1)
            par[:fhs[l], 323 + l] = aW[0, fhs[l]:2 * fhs[l]]
        m["params"] = par
        in_maps.append(m)
    return in_maps


def kernel(x, adj, W0, b0, aW0, ab0, W1, b1, aW1, ab1, W2, b2, aW2, ab2):
    inputs = dict(x=x, adj=adj, W0=W0, b0=b0, aW0=aW0, ab0=ab0,
                  W1=W1, b1=b1, aW1=aW1, ab1=ab1, W2=W2, b2=b2, aW2=aW2, ab2=ab2)
    ab = [float(np.asarray(inputs[f"ab{l}"]).reshape(-1)[0]) for l in range(3)]
    nc = _get_program(ab)
    in_maps = _make_in_maps(inputs)
    res = run_bass_kernel_spmd(nc, in_maps, list(range(N_CORES)))
    out = np.concatenate([res.results[c]["out"] for c in range(N_CORES)], axis=0)
    return out.astype(np.float32)


# revision 16
# speedup vs baseline: 1.1057x; 1.1057x over previous
"""GAT-style 3-layer attention graph network on 8 TRN2 NeuronCores.

Math: per layer, alpha[i,j] = adj[i,j]*exp(el[i]+er[j]+ab) / sum_k adj[i,k]*exp(el[i]+er[k]+ab)
The exp(el[i]) factor cancels between numerator and denominator, so with
w[j] = exp(er[j]+ab):
    out[i] = relu( (sum_j adj[i,j]*w[j]*h[j]) / (sum_j adj[i,j]*w[j]) )
i.e. one [N,N]@[N,F+1] matmul per layer against G = [h*w | w], with adj
constant across layers.

Distribution: row-shard adj across the 8 cores (1024 dest rows each). adj is
0/1 so it is exactly representable in fp8_e4m3: the host pre-transposes each
core's row-block into the matmul lhsT tile layout [128, m, k, 128] fp8
(the PE contracts over the partition index, which for the aggregation is
adj's column index), and it stays SBUF-resident (8MB/core) across all 3
layers; the mixed fp8-lhsT x fp16-rhs matmul is exact for 0/1 weights.
Each layer all-gathers the 8192x(F+1) fp16 G matrix (2MB) in two node-halves
so the first gather overlaps the previous layer's aggregation, and the next
layer's G is built inside the per-m epilogue of the current aggregation.
"""
import numpy as np

import concourse.bass as bass
import concourse.mybir as mybir
import concourse.tile as tile
from concourse.masks import make_identity
from concourse.tile_rust import add_dep_helper
from concourse.bass_utils import run_bass_kernel_spmd

F32 = mybir.dt.float32
F16 = mybir.dt.float16  # G storage dtype: 10-bit mantissa
F8 = mybir.dt.float8e4   # adj storage: 0/1 exact in fp8_e4m3, 4x weight-load

N_CORES = 8
N = 8192
NL = N // N_CORES          # 1024 local dest rows per core
NT = NL // 128             # 8 local node tiles
KT = N // 128              # 64 contraction tiles
LEAK = 0.2


def _split_excess_waits(nc, max_waits=1):
    """This walrus build allows only one sync-wait command per instruction;
    split any instruction carrying more into preceding single-wait nops."""
    n_split = 0
    for fn in nc.m.functions:
        for bb in fn.blocks:
            insts = bb.instructions
            i = 0
            while i < len(insts):
                inst = insts[i]
                si = inst.sync_info
                if si is not None and len(si.on_wait) > max_waits:
                    waits = list(si.on_wait)
                    extra, keep = waits[:-max_waits], waits[-max_waits:]
                    nops = []
                    for j, w in enumerate(extra):
                        nop = mybir.InstNoOp(
                            name=f"{inst.name}-waitsplit-{j}", ins=[], outs=[]
                        )
                        nop.engine = inst.engine
                        nop.sync_info = mybir.SyncInfo(on_wait=[w], on_update=[])
                        nops.append(nop)
                    inst.sync_info = mybir.SyncInfo(
                        on_wait=keep, on_update=list(si.on_update)
                    )
                    insts[i:i] = nops
                    i += len(nops)
                    n_split += 1
                i += 1
    return n_split


def _build_program(ab, for_sim=False):
    """ab: the three attention bias floats (baked in as memset constants)."""
    fhs = [128, 128, 64]  # per-layer linear output width

    nc = bass.Bass(num_devices=N_CORES)

    adj_ext = nc.dram_tensor("adjt", [128, NT, KT, 128], F8, kind="ExternalInput")
    x_ext = nc.dram_tensor("xt_local", [128, NL], F32, kind="ExternalInput")
    # packed params: cols [0:128)=w0t [128:256)=w1t [256:320)=w2t,
    # 320+l = b_l column, 323+l = awr_l column (rows past fh zero-padded)
    par_ext = nc.dram_tensor("params", [128, 326], F32, kind="ExternalInput")
    out_ext = nc.dram_tensor("out", [NL, 64], F32, kind="ExternalOutput")

    # all-gather payload in tiled layout, split in two node-halves per layer
    # so the first gather overlaps the previous aggregation: half h of layer l
    # holds rank blocks [128, 4*(fh+1)] with (p, t, f) = G[c*1024+(4h+t)*128+p, f]
    ag_ext = [[nc.dram_tensor(f"ag{l}h{h}", [N_CORES * 128, NT // 2 * (fhs[l] + 1)],
                              F16, addr_space="Shared") for h in range(2)]
              for l in range(3)]

    with tile.TileContext(nc) as tc:
        with (
            tc.tile_pool(name="const", bufs=1) as cp,
            tc.tile_pool(name="adjt", bufs=1) as ap_,
            tc.tile_pool(name="slabs", bufs=3) as sp,
            tc.tile_pool(name="gsb", bufs=2) as gp,
            tc.tile_pool(name="misc", bufs=2) as mp,
            tc.tile_pool(name="gloc", bufs=2) as glp,
            tc.tile_pool(name="dram", bufs=3, space="DRAM") as dp,
            tc.tile_pool(name="ptf32", bufs=2, space="PSUM") as ptf32,
            tc.tile_pool(name="plin", bufs=2, space="PSUM") as plin,
            tc.tile_pool(name="per", bufs=2, space="PSUM") as per,
            tc.tile_pool(name="pbig", bufs=2, space="PSUM") as pbig,
        ):
            # ---- constants / params ----
            ident_f32 = cp.tile([128, 128], F32)
            make_identity(nc, ident_f32[:])
            par = cp.tile([128, 326], F32)
            nc.sync.dma_start(out=par[:], in_=par_ext.ap())
            woff = [0, 128, 256]
            wt_sb = [par[:, woff[l]:woff[l] + fhs[l]] for l in range(3)]
            b_sb = [par[0:fhs[l], 320 + l:321 + l] for l in range(3)]
            awr_sb = [par[0:fhs[l], 323 + l:324 + l] for l in range(3)]
            ab_sb = []
            for l in range(3):
                t = cp.tile([128, 1], F32, tag=f"ab{l}")
                nc.gpsimd.memset(t[:], float(ab[l]))
                ab_sb.append(t)

            # ---- x arrives pre-transposed: [fi, node] ----
            curT = sp.tile([128, NL], F32, tag="slab")
            nc.sync.dma_start(out=curT[:], in_=x_ext.ap())

            # ---- adj arrives pre-transposed+tiled from host: [128, m, k, 128]
            # f16; tile (k, m) = adj[m-block rows, k-block cols].T. Load in
            # m-pair chunks so layer-0 m-chains can start after ~1/4 the DMA.
            adjT = ap_.tile([128, NT, KT, 128], F8)
            adjt_insts = []
            for d in range(NT):
                adjt_insts.append(nc.gpsimd.dma_start(
                    out=adjT[:, d, :, :],
                    in_=adj_ext[:, d, :, :],
                ))

            # ---- G-prep helper: one 128-node block of layer l's G ----
            # src_col: [128(fi), 128] column of transposed prev activations
            def prep_block(l, src_col, gl, m):
                fh = fhs[l]
                pl = plin.tile([128, 128], F32, tag="lin")
                nc.tensor.matmul(pl[0:fh, 0:128], wt_sb[l], src_col,
                                 start=True, stop=True)
                hcol = mp.tile([128, 128], F32, tag="hcol")
                nc.scalar.activation(
                    hcol[0:fh, :], pl[0:fh, 0:128],
                    mybir.ActivationFunctionType.Prelu,
                    bias=b_sb[l], scale=1.0, alpha=LEAK,
                )
                pe_ = per.tile([128, 1], F32, tag="er")
                nc.tensor.matmul(pe_[:, 0:1], hcol[0:fh, :], awr_sb[l],
                                 start=True, stop=True)
                ec = mp.tile([128, 1], F32, tag="expc")
                nc.scalar.activation(
                    ec[:], pe_[:, 0:1], mybir.ActivationFunctionType.Exp,
                    bias=ab_sb[l][:], scale=1.0,
                )
                ptg = ptf32.tile([128, 128], F32, tag="ptf")
                nc.tensor.transpose(ptg[:, 0:fh], hcol[0:fh, :],
                                    ident_f32[0:fh, 0:fh])
                nc.vector.tensor_scalar_mul(gl[:, m, 0:fh], ptg[:, 0:fh], ec[:])
                nc.vector.tensor_copy(gl[:, m, fh:fh + 1], ec[:])

            gsb_tiles = {}

            def fire_gather(l, gl, h):
                """All-gather node-half h of layer l's local G block, then
                immediately queue the SBUF reload of that half (so it sits
                before the next gld store in the SP HWDGE FIFO)."""
                fh = fhs[l]
                gld = dp.tile([128, NT // 2 * (fh + 1)], F16, tag="gld")
                nc.sync.dma_start(
                    out=gld[:], in_=gl[:, h * (NT // 2):(h + 1) * (NT // 2), :]
                )
                if for_sim:
                    nc.sync.dma_start(out=ag_ext[l][h][0:128, :], in_=gld[:])
                else:
                    nc.gpsimd.collective_compute(
                        "AllGather", mybir.AluOpType.bypass,
                        replica_groups=[list(range(N_CORES))],
                        ins=[gld.opt()], outs=[ag_ext[l][h].ap().opt()],
                    )
                if h == 0:
                    gsb_new = gp.tile([128, N_CORES, NT, fh + 1], F16, tag="gsb")
                    gsb_tiles[l] = gsb_new
                return nc.sync.dma_start(
                    out=gsb_tiles[l][:, :, h * (NT // 2):(h + 1) * (NT // 2), :],
                    in_=ag_ext[l][h].ap().rearrange(
                        "(c p) (t f) -> p c t f", p=128, f=fh + 1
                    ),
                )

            # ---- layer 0 G from x (overlaps the adj load) ----
            gl_cur = glp.tile([128, NT, fhs[0] + 1], F16, tag="gloc")
            for m in range(NT):
                prep_block(0, curT[:, m * 128:(m + 1) * 128], gl_cur, m)
                if m == NT // 2 - 1:
                    fire_gather(0, gl_cur, 0)
            g0h2 = fire_gather(0, gl_cur, 1)
            # let layer 0's G reload jump ahead of the bulk of the adj load:
            # chunks 2+ aren't needed until their m-chains run anyway
            for d in range(2, NT):
                add_dep_helper(adjt_insts[d].ins, g0h2.ins, sync=True,
                               reason="adjt bulk yields to L0 G reload")

            # ---- layers: all-gather G, aggregate, and build next layer's G
            # inside the per-m epilogue so only the collective + G reload sit
            # on the layer boundary ----
            for l in range(3):
                fh = fhs[l]
                # gathered G halves were queued by fire_gather; k = c*NT + t,
                # half h covers k with (k % NT) in [4h, 4h+4)
                gsb = gsb_tiles[l]
                if l < 2:
                    gl_next = glp.tile([128, NT, fhs[l + 1] + 1], F16, tag="gloc")
                else:
                    ostage = mp.tile([128, NT, 64], F32, tag="ostage")

                # epilogue (+ next-layer G prep) for block m, emitted one
                # m-iteration late so the PE's static order interleaves the
                # small prep ops between big-MM chains without stalling
                def epilogue(m, bp):
                    recip = mp.tile([128, 1], F32, tag="recip")
                    nc.vector.reciprocal(recip[:], bp[:, fh:fh + 1])
                    if l < 2:
                        h2 = mp.tile([128, fh], F32, tag="h2")
                        nc.scalar.activation(
                            h2[:], bp[:, 0:fh], mybir.ActivationFunctionType.Relu,
                            bias=0.0, scale=recip[:],
                        )
                        pt = ptf32.tile([128, 128], F32, tag="ptf")
                        nc.tensor.transpose(pt[:, 0:128], h2[:], ident_f32[:])
                        cpcol = mp.tile([128, 128], F32, tag="cpcol")
                        nc.vector.tensor_copy(cpcol[:], pt[:, 0:128])
                        prep_block(l + 1, cpcol[:], gl_next, m)
                    else:
                        nc.scalar.activation(
                            ostage[:, m, :], bp[:, 0:fh],
                            mybir.ActivationFunctionType.Relu,
                            bias=0.0, scale=recip[:],
                        )

                ks = [k for k in range(KT) if k % NT < NT // 2] + \
                     [k for k in range(KT) if k % NT >= NT // 2]
                pending = None
                for m in range(NT):
                    bp = pbig.tile([128, fh + 1], F32, tag="big")
                    for i, k in enumerate(ks):
                        nc.tensor.matmul(
                            bp[:],
                            adjT[:, m, k, :],
                            gsb[:, k // NT, k % NT, :],
                            start=(i == 0), stop=(i == KT - 1),
                        )
                    if pending is not None:
                        epilogue(*pending)
                        if l < 2 and pending[0] == NT // 2 - 1:
                            fire_gather(l + 1, gl_next, 0)
                    pending = (m, bp)
                epilogue(*pending)
                if l < 2:
                    fire_gather(l + 1, gl_next, 1)
                    gl_cur = gl_next
                else:
                    nc.sync.dma_start(
                        out=out_ext.ap().rearrange("(m p) f -> p m f", p=128),
                        in_=ostage[:],
                    )

    _split_excess_waits(nc)
    return nc


_PROG_CACHE = {}


def _get_program(ab):
    key = tuple(round(a, 9) for a in ab)
    if key not in _PROG_CACHE:
        _PROG_CACHE[key] = _build_program(ab)
    return _PROG_CACHE[key]


def _make_in_maps(inputs):
    """Build the per-core input maps from the full (unsharded) input dict."""
    fhs = [128, 128, 64]
    x = np.asarray(inputs["x"], np.float32)
    adj = np.asarray(inputs["adj"], np.float32)
    in_maps = []
    for c in range(N_CORES):
        import ml_dtypes
        blk = adj[c * NL:(c + 1) * NL, :].astype(ml_dtypes.float8_e4m3)
        # [NL, N] -> [m, q, k, p] -> lhsT tile layout [p, m, k, q]
        adjt = blk.reshape(NT, 128, KT, 128).transpose(3, 0, 2, 1)
        m = {
            "adjt": np.ascontiguousarray(adjt),
            "xt_local": np.ascontiguousarray(x[c * NL:(c + 1) * NL, :].T),
        }
        par = np.zeros((128, 326), np.float32)
        woff = [0, 128, 256]
        for l in range(3):
            W = np.asarray(inputs[f"W{l}"], np.float32)
            b = np.asarray(inputs[f"b{l}"], np.float32)
            aW = np.asarray(inputs[f"aW{l}"], np.float32)
            par[:, woff[l]:woff[l] + fhs[l]] = W.T
            par[:fhs[l], 320 + l] = b.reshape(-1)
            par[:fhs[l], 323 + l] = aW[0, fhs[l]:2 * fhs[l]]
        m["params"] = par
        in_maps.append(m)
    return in_maps


def kernel(x, adj, W0, b0, aW0, ab0, W1, b1, aW1, ab1, W2, b2, aW2, ab2):
    inputs = dict(x=x, adj=adj, W0=W0, b0=b0, aW0=aW0, ab0=ab0,
                  W1=W1, b1=b1, aW1=aW1, ab1=ab1, W2=W2, b2=b2, aW2=aW2, ab2=ab2)
    ab = [float(np.asarray(inputs[f"ab{l}"]).reshape(-1)[0]) for l in range(3)]
    nc = _get_program(ab)
    in_maps = _make_in_maps(inputs)
    res = run_bass_kernel_spmd(nc, in_maps, list(range(N_CORES)))
    out = np.concatenate([res.results[c]["out"] for c in range(N_CORES)], axis=0)
    return out.astype(np.float32)


# revision 19
# speedup vs baseline: 1.1269x; 1.0192x over previous
"""GAT-style 3-layer attention graph network on 8 TRN2 NeuronCores.

Math: per layer, alpha[i,j] = adj[i,j]*exp(el[i]+er[j]+ab) / sum_k adj[i,k]*exp(el[i]+er[k]+ab)
The exp(el[i]) factor cancels between numerator and denominator, so with
w[j] = exp(er[j]+ab):
    out[i] = relu( (sum_j adj[i,j]*w[j]*h[j]) / (sum_j adj[i,j]*w[j]) )
i.e. one [N,N]@[N,F+1] matmul per layer against G = [h*w | w], with adj
constant across layers.

Distribution: row-shard adj across the 8 cores (1024 dest rows each). adj is
0/1 so it is exactly representable in fp8_e4m3: the host pre-transposes each
core's row-block into the matmul lhsT tile layout [128, m, k, 128] fp8
(the PE contracts over the partition index, which for the aggregation is
adj's column index), and it stays SBUF-resident (8MB/core) across all 3
layers; the mixed fp8-lhsT x fp16-rhs matmul is exact for 0/1 weights.
Each layer all-gathers the 8192x(F+1) fp16 G matrix (2MB) in two node-halves
so the first gather overlaps the previous layer's aggregation, and the next
layer's G is built inside the per-m epilogue of the current aggregation.
"""
import numpy as np

import concourse.bass as bass
import concourse.mybir as mybir
import concourse.tile as tile
from concourse.masks import make_identity
from concourse.tile_rust import add_dep_helper
from concourse.bass_utils import run_bass_kernel_spmd

F32 = mybir.dt.float32
F16 = mybir.dt.float16  # G storage dtype: 10-bit mantissa
F8 = mybir.dt.float8e4   # adj storage: 0/1 exact in fp8_e4m3, 4x weight-load

N_CORES = 8
N = 8192
NL = N // N_CORES          # 1024 local dest rows per core
NT = NL // 128             # 8 local node tiles
KT = N // 128              # 64 contraction tiles
LEAK = 0.2


def _split_excess_waits(nc, max_waits=1):
    """This walrus build allows only one sync-wait command per instruction;
    split any instruction carrying more into preceding single-wait nops."""
    n_split = 0
    for fn in nc.m.functions:
        for bb in fn.blocks:
            insts = bb.instructions
            i = 0
            while i < len(insts):
                inst = insts[i]
                si = inst.sync_info
                if si is not None and len(si.on_wait) > max_waits:
                    waits = list(si.on_wait)
                    extra, keep = waits[:-max_waits], waits[-max_waits:]
                    nops = []
                    for j, w in enumerate(extra):
                        nop = mybir.InstNoOp(
                            name=f"{inst.name}-waitsplit-{j}", ins=[], outs=[]
                        )
                        nop.engine = inst.engine
                        nop.sync_info = mybir.SyncInfo(on_wait=[w], on_update=[])
                        nops.append(nop)
                    inst.sync_info = mybir.SyncInfo(
                        on_wait=keep, on_update=list(si.on_update)
                    )
                    insts[i:i] = nops
                    i += len(nops)
                    n_split += 1
                i += 1
    return n_split


def _build_program(ab, for_sim=False):
    """ab: the three attention bias floats (baked in as memset constants)."""
    fhs = [128, 128, 64]  # per-layer linear output width

    nc = bass.Bass(num_devices=N_CORES)

    adj_ext = nc.dram_tensor("adjt", [128, NT, KT, 128], F8, kind="ExternalInput")
    x_ext = nc.dram_tensor("xt_local", [128, NL], F32, kind="ExternalInput")
    # packed params: cols [0:128)=w0t [128:256)=w1t [256:320)=w2t,
    # 320+l = b_l column, 323+l = awr_l column (rows past fh zero-padded)
    par_ext = nc.dram_tensor("params", [128, 326], F32, kind="ExternalInput")
    out_ext = nc.dram_tensor("out", [NL, 64], F32, kind="ExternalOutput")

    # all-gather payload in tiled layout, split in two node-halves per layer
    # so the first gather overlaps the previous aggregation: half h of layer l
    # holds rank blocks [128, 4*(fh+1)] with (p, t, f) = G[c*1024+(4h+t)*128+p, f]
    ag_ext = [[nc.dram_tensor(f"ag{l}h{h}", [N_CORES * 128, NT // 2 * (fhs[l] + 1)],
                              F16, addr_space="Shared") for h in range(2)]
              for l in range(3)]

    with tile.TileContext(nc) as tc:
        with (
            tc.tile_pool(name="const", bufs=1) as cp,
            tc.tile_pool(name="adjt", bufs=1) as ap_,
            tc.tile_pool(name="slabs", bufs=3) as sp,
            tc.tile_pool(name="gsb", bufs=2) as gp,
            tc.tile_pool(name="misc", bufs=2) as mp,
            tc.tile_pool(name="gloc", bufs=2) as glp,
            tc.tile_pool(name="dram", bufs=3, space="DRAM") as dp,
            tc.tile_pool(name="ptf32", bufs=2, space="PSUM") as ptf32,
            tc.tile_pool(name="plin", bufs=1, space="PSUM") as plin,
            tc.tile_pool(name="per", bufs=1, space="PSUM") as per,
            tc.tile_pool(name="pbig", bufs=4, space="PSUM") as pbig,
        ):
            # ---- constants / params ----
            ident_f32 = cp.tile([128, 128], F32)
            make_identity(nc, ident_f32[:])
            par = cp.tile([128, 326], F32)
            nc.sync.dma_start(out=par[:], in_=par_ext.ap())
            woff = [0, 128, 256]
            wt_sb = [par[:, woff[l]:woff[l] + fhs[l]] for l in range(3)]
            b_sb = [par[0:fhs[l], 320 + l:321 + l] for l in range(3)]
            awr_sb = [par[0:fhs[l], 323 + l:324 + l] for l in range(3)]
            ab_sb = []
            for l in range(3):
                t = cp.tile([128, 1], F32, tag=f"ab{l}")
                nc.gpsimd.memset(t[:], float(ab[l]))
                ab_sb.append(t)

            # ---- x arrives pre-transposed: [fi, node] ----
            curT = sp.tile([128, NL], F32, tag="slab")
            nc.sync.dma_start(out=curT[:], in_=x_ext.ap())

            # ---- adj arrives pre-transposed+tiled from host: [128, m, k, 128]
            # f16; tile (k, m) = adj[m-block rows, k-block cols].T. Load in
            # m-pair chunks so layer-0 m-chains can start after ~1/4 the DMA.
            adjT = ap_.tile([128, NT, KT, 128], F8)
            adjt_insts = []
            for d in range(NT):
                adjt_insts.append(nc.gpsimd.dma_start(
                    out=adjT[:, d, :, :],
                    in_=adj_ext[:, d, :, :],
                ))

            # ---- G-prep helper: one 128-node block of layer l's G ----
            # src_col: [128(fi), 128] column of transposed prev activations
            def prep_block(l, src_col, gl, m):
                fh = fhs[l]
                pl = plin.tile([128, 128], F32, tag="lin")
                nc.tensor.matmul(pl[0:fh, 0:128], wt_sb[l], src_col,
                                 start=True, stop=True)
                hcol = mp.tile([128, 128], F32, tag="hcol")
                nc.scalar.activation(
                    hcol[0:fh, :], pl[0:fh, 0:128],
                    mybir.ActivationFunctionType.Prelu,
                    bias=b_sb[l], scale=1.0, alpha=LEAK,
                )
                pe_ = per.tile([128, 1], F32, tag="er")
                nc.tensor.matmul(pe_[:, 0:1], hcol[0:fh, :], awr_sb[l],
                                 start=True, stop=True)
                ec = mp.tile([128, 1], F32, tag="expc")
                nc.scalar.activation(
                    ec[:], pe_[:, 0:1], mybir.ActivationFunctionType.Exp,
                    bias=ab_sb[l][:], scale=1.0,
                )
                ptg = ptf32.tile([128, 128], F32, tag="ptf")
                nc.tensor.transpose(ptg[:, 0:fh], hcol[0:fh, :],
                                    ident_f32[0:fh, 0:fh])
                nc.vector.tensor_scalar_mul(gl[:, m, 0:fh], ptg[:, 0:fh], ec[:])
                nc.vector.tensor_copy(gl[:, m, fh:fh + 1], ec[:])

            gsb_tiles = {}

            def fire_gather(l, gl, h):
                """All-gather node-half h of layer l's local G block, then
                immediately queue the SBUF reload of that half (so it sits
                before the next gld store in the SP HWDGE FIFO)."""
                fh = fhs[l]
                gld = dp.tile([128, NT // 2 * (fh + 1)], F16, tag="gld")
                nc.sync.dma_start(
                    out=gld[:], in_=gl[:, h * (NT // 2):(h + 1) * (NT // 2), :]
                )
                if for_sim:
                    nc.sync.dma_start(out=ag_ext[l][h][0:128, :], in_=gld[:])
                else:
                    nc.gpsimd.collective_compute(
                        "AllGather", mybir.AluOpType.bypass,
                        replica_groups=[list(range(N_CORES))],
                        ins=[gld.opt()], outs=[ag_ext[l][h].ap().opt()],
                    )
                if h == 0:
                    gsb_new = gp.tile([128, N_CORES, NT, fh + 1], F16, tag="gsb")
                    gsb_tiles[l] = gsb_new
                return nc.sync.dma_start(
                    out=gsb_tiles[l][:, :, h * (NT // 2):(h + 1) * (NT // 2), :],
                    in_=ag_ext[l][h].ap().rearrange(
                        "(c p) (t f) -> p c t f", p=128, f=fh + 1
                    ),
                )

            # ---- layer 0 G from x (overlaps the adj load) ----
            gl_cur = glp.tile([128, NT, fhs[0] + 1], F16, tag="gloc")
            for m in range(NT):
                prep_block(0, curT[:, m * 128:(m + 1) * 128], gl_cur, m)
                if m == NT // 2 - 1:
                    fire_gather(0, gl_cur, 0)
            g0h2 = fire_gather(0, gl_cur, 1)
            # let layer 0's G reload jump ahead of the bulk of the adj load:
            # chunks 2+ aren't needed until their m-chains run anyway
            for d in range(2, NT):
                add_dep_helper(adjt_insts[d].ins, g0h2.ins, sync=True,
                               reason="adjt bulk yields to L0 G reload")

            # ---- layers: all-gather G, aggregate, and build next layer's G
            # inside the per-m epilogue so only the collective + G reload sit
            # on the layer boundary ----
            # ---- layers: the epilogue of block m (which also builds the
            # NEXT layer's G block m and fires its gathers) is flushed after
            # the FOLLOWING big-MM chain is emitted -- including across the
            # layer seam -- so the PE never stalls on the small-op chains ----
            def make_epilogue(l, gl_next, ostage):
                fh = fhs[l]

                def epilogue(m, bp):
                    recip = mp.tile([128, 1], F32, tag="recip")
                    nc.vector.reciprocal(recip[:], bp[:, fh:fh + 1])
                    if l < 2:
                        h2 = mp.tile([128, fh], F32, tag="h2")
                        nc.scalar.activation(
                            h2[:], bp[:, 0:fh], mybir.ActivationFunctionType.Relu,
                            bias=0.0, scale=recip[:],
                        )
                        pt = ptf32.tile([128, 128], F32, tag="ptf")
                        nc.tensor.transpose(pt[:, 0:128], h2[:], ident_f32[:])
                        cpcol = mp.tile([128, 128], F32, tag="cpcol")
                        nc.vector.tensor_copy(cpcol[:], pt[:, 0:128])
                        prep_block(l + 1, cpcol[:], gl_next, m)
                        if m == NT // 2 - 1:
                            fire_gather(l + 1, gl_next, 0)
                        elif m == NT - 1:
                            fire_gather(l + 1, gl_next, 1)
                    else:
                        nc.scalar.activation(
                            ostage[:, m, :], bp[:, 0:fh],
                            mybir.ActivationFunctionType.Relu,
                            bias=0.0, scale=recip[:],
                        )
                        if m == NT - 1:
                            nc.sync.dma_start(
                                out=out_ext.ap().rearrange(
                                    "(m p) f -> p m f", p=128),
                                in_=ostage[:],
                            )

                return epilogue

            pending = None
            for l in range(3):
                fh = fhs[l]
                gsb = gsb_tiles[l]
                if l < 2:
                    gl_next = glp.tile([128, NT, fhs[l + 1] + 1], F16, tag="gloc")
                    ostage = None
                else:
                    gl_next = None
                    ostage = mp.tile([128, NT, 64], F32, tag="ostage")
                epi = make_epilogue(l, gl_next, ostage)
                # k = c*NT + t; gather-half 0 covers (k % NT) < NT/2
                ks = [k for k in range(KT) if k % NT < NT // 2] + \
                     [k for k in range(KT) if k % NT >= NT // 2]
                for m in range(NT):
                    bp = pbig.tile([128, fh + 1], F32, tag="big")
                    for i, k in enumerate(ks):
                        if i == KT // 2 and pending is not None:
                            # flush the previous block's epilogue between the
                            # two k-halves: its G-half-2 gather (for the seam
                            # case) is then traced before any MM that reads it
                            pending[0](pending[1], pending[2])
                            pending = None
                        nc.tensor.matmul(
                            bp[:],
                            adjT[:, m, k, :],
                            gsb[:, k // NT, k % NT, :],
                            start=(i == 0), stop=(i == KT - 1),
                        )
                    if pending is not None:
                        pending[0](pending[1], pending[2])
                    pending = (epi, m, bp)
            pending[0](pending[1], pending[2])

    _split_excess_waits(nc)
    return nc


_PROG_CACHE = {}


def _get_program(ab):
    key = tuple(round(a, 9) for a in ab)
    if key not in _PROG_CACHE:
        _PROG_CACHE[key] = _build_program(ab)
    return _PROG_CACHE[key]


def _make_in_maps(inputs):
    """Build the per-core input maps from the full (unsharded) input dict."""
    fhs = [128, 128, 64]
    x = np.asarray(inputs["x"], np.float32)
    adj = np.asarray(inputs["adj"], np.float32)
    in_maps = []
    for c in range(N_CORES):
        import ml_dtypes
        blk = adj[c * NL:(c + 1) * NL, :].astype(ml_dtypes.float8_e4m3)
        # [NL, N] -> [m, q, k, p] -> lhsT tile layout [p, m, k, q]
        adjt = blk.reshape(NT, 128, KT, 128).transpose(3, 0, 2, 1)
        m = {
            "adjt": np.ascontiguousarray(adjt),
            "xt_local": np.ascontiguousarray(x[c * NL:(c + 1) * NL, :].T),
        }
        par = np.zeros((128, 326), np.float32)
        woff = [0, 128, 256]
        for l in range(3):
            W = np.asarray(inputs[f"W{l}"], np.float32)
            b = np.asarray(inputs[f"b{l}"], np.float32)
            aW = np.asarray(inputs[f"aW{l}"], np.float32)
            par[:, woff[l]:woff[l] + fhs[l]] = W.T
            par[:fhs[l], 320 + l] = b.reshape(-1)
            par[:fhs[l], 323 + l] = aW[0, fhs[l]:2 * fhs[l]]
        m["params"] = par
        in_maps.append(m)
    return in_maps


def kernel(x, adj, W0, b0, aW0, ab0, W1, b1, aW1, ab1, W2, b2, aW2, ab2):
    inputs = dict(x=x, adj=adj, W0=W0, b0=b0, aW0=aW0, ab0=ab0,
                  W1=W1, b1=b1, aW1=aW1, ab1=ab1, W2=W2, b2=b2, aW2=aW2, ab2=ab2)
    ab = [float(np.asarray(inputs[f"ab{l}"]).reshape(-1)[0]) for l in range(3)]
    nc = _get_program(ab)
    in_maps = _make_in_maps(inputs)
    res = run_bass_kernel_spmd(nc, in_maps, list(range(N_CORES)))
    out = np.concatenate([res.results[c]["out"] for c in range(N_CORES)], axis=0)
    return out.astype(np.float32)
